# revision 34
# baseline (speedup 1.0000x reference)
"""Cosine-similarity causal attention (B=4, S=2048, D=768, H=12) on 8 TRN2 cores.

Wire-optimized: the axon PJRT tunnel moves ~52 MB/s, so host<->device bytes
dominate the spmd-call wall time, not on-device compute (~0.3 ms). Per core we
ship only: half of its batch's x (the pair partner holds the other half; an
on-chip pair AllGather restores the full sequence), a quarter of its
head-group's weight bundle (on-chip 4-way AllGather across the same-group
cores restores it), and 264 B of temperature. Causal masks and selector
constants are generated on device (memset + affine_select). The two
per-head-group partial outputs are summed on-chip with a pair ReduceScatter
and each core returns one half of its batch's output uint8-quantized with
per-row f32 scales packed into the same tensor (error <= rowmax/253).

Sharding: core c = 2b+g -> batch b (4), head-group g (2 groups of 6 heads).

Compute layout per core (matmul-packed, unchanged):
  xT    [768, 2048]  x transposed -> contraction dim on partitions
  q/k   kept transposed [384, 2048] (head dim on partitions) -> direct
        lhsT/rhs for the scores matmul; row norms via ones-block matmuls.
  v     natural [2048, 384] + interleaved ones column per head (width 65) so
        the attention-value matmul also produces the softmax denominator.
  scores computed transposed [k, q]; softmax needs no max-subtraction
        because cosine scores are bounded by |temperature|.
"""

import numpy as np
from contextlib import ExitStack

import concourse.bass as bass
import concourse.mybir as mybir
import concourse.tile as tile
from concourse import bacc, bass_utils

B, S, D, H, HD = 4, 2048, 768, 12, 64
NCORES, NGRP = 8, 2
HPC = H // NGRP          # 6 heads per core
DPC = HPC * HD           # 384 head-dims per core
SH = S // NGRP           # 1024 = per-core x / y half
SEG = 512                # q-chunk width
NSEG = S // SEG          # 4
NB = S // 128            # 16 key blocks
FCH = D // 128           # 6 contraction chunks for projections
MCH = DPC // 128         # 3 head-dim chunks (2 heads each)
VW = HD + 1              # 65 = value width per head incl. the ones column
CPACK = 452              # f32 const columns (blkones|sel26|_|temp|eps)
WROW = D * DPC           # 294912 = elements per weight-bundle quarter
XSPL = 3                 # x upload split (parallel tunnel streams)
WSPL = 1                 # w upload split (tested 2: extra-arg overhead cancels the gain)

F32 = mybir.dt.float32
BF16 = mybir.dt.bfloat16
import os
import ml_dtypes
if os.environ.get("COSATT_DT", "bf16") == "bf16":
    DT = mybir.dt.bfloat16   # matmul operand dtype
    NPDT = ml_dtypes.bfloat16
else:
    DT = mybir.dt.float32
    NPDT = np.float32
AF = mybir.ActivationFunctionType
MUL = mybir.AluOpType.mult

W_GROUPS = [[0, 2, 4, 6], [1, 3, 5, 7]]   # same head-group cores
P_GROUPS = [[0, 1], [2, 3], [4, 5], [6, 7]]  # same-batch pairs

_CACHE: dict = {}


def _build_nc():
    nc = bacc.Bacc(
        "TRN2",
        target_bir_lowering=False,
        debug=False,
        enable_asserts=False,
        num_devices=NCORES,
    )

    # x split into a few pieces: the axon tunnel transfers separate args in
    # parallel streams, so no single stream should dominate the upload
    xhs = [nc.dram_tensor(f"xh{i}", [D // XSPL, SH], DT, kind="ExternalInput").ap()
           for i in range(XSPL)]
    # w bundle carries 66 bf16 temperature values appended past the gathered
    # region (temps are per-group, read from the local bounce, not gathered;
    # temperature in bf16 costs <=0.4% on scores, and the reference's 1.0 is
    # exact) — saves a whole jit argument (~15-40 ms of per-arg overhead)
    w8 = nc.dram_tensor("w8", [1, WROW + 66], DT, kind="ExternalInput").ap()
    # y ships uint8-quantized with a per-row f32 scale packed into the last
    # 4 columns (halves both the donated-zeros upload and the fetch):
    #   u = round(y * 126.5/rowmax + 128); decode (u-128)*rowmax/126.5
    # (the DVE f32->uint8 copy rounds to nearest; error <= 0.5 LSB of rowmax/126.5)
    y = nc.dram_tensor("y", [SH, D + 4], mybir.dt.uint8,
                       kind="ExternalOutput").ap()
    y_f32 = y.bitcast(F32)  # [SH, (D+4)/4]; scale lives at f32 col D//4

    with tile.TileContext(nc) as tc, ExitStack() as ctx:
        cpool = ctx.enter_context(tc.tile_pool(name="const", bufs=1))
        big = ctx.enter_context(tc.tile_pool(name="big", bufs=1))
        dpool = ctx.enter_context(tc.tile_pool(name="dram", bufs=1, space="DRAM"))

        # ---------------- on-chip input gather ----------------
        # Bounce I/O tensors through internal DRAM (collectives cannot
        # address kernel I/O directly), then reassemble: 4-way AllGather for
        # the weight bundle, pair AllGather for the sequence halves.
        w_inb = dpool.tile([1, WROW], DT, tag="w_inb", name="w_inb")
        wall = dpool.tile([4 * D, DPC], DT, tag="wall", name="wall")
        x_inb = dpool.tile([D, SH], DT, tag="x_inb", name="x_inb")
        xall = dpool.tile([2 * D, SH], DT, tag="xall", name="xall")
        nc.sync.dma_start(w_inb[:], w8[:, 0:WROW])
        XCH = D // XSPL
        for i in range(XSPL):
            nc.sync.dma_start(x_inb[i * XCH:(i + 1) * XCH, :], xhs[i][:])
        nc.gpsimd.collective_compute(
            "AllGather", mybir.AluOpType.bypass, replica_groups=W_GROUPS,
            ins=[w_inb.opt()], outs=[wall.opt()])
        nc.gpsimd.collective_compute(
            "AllGather", mybir.AluOpType.bypass, replica_groups=P_GROUPS,
            ins=[x_inb.opt()], outs=[xall.opt()])

        # ---------------- constants, generated on device ----------------
        cf = cpool.tile([128, 1024], F32, tag="cf", name="cf")
        nc.vector.memset(cf[:, 0:CPACK], 0.0)
        nc.vector.memset(cf[0:64, 0:1], 1.0)      # blkones col 0
        nc.vector.memset(cf[64:128, 1:2], 1.0)    # blkones col 1
        for m in range(MCH):
            # sel26 block m: row 32m one-hot over cols 0-63, row 32m+1 over
            # cols 64-127, zero elsewhere. Engine ops need 32-aligned start
            # partitions, so carve it from ones with two affine selects:
            # keep iff 0 <= f - 64*(p - 32m) <= 63.
            blk = cf[0:66, 2 + 128 * m:2 + 128 * (m + 1)]
            nc.vector.memset(blk, 1.0)
            nc.gpsimd.affine_select(
                blk, blk, pattern=[[1, 128]],
                compare_op=mybir.AluOpType.is_ge, fill=0.0,
                base=2048 * m, channel_multiplier=-64)
            nc.gpsimd.affine_select(
                blk, blk, pattern=[[-1, 128]],
                compare_op=mybir.AluOpType.is_ge, fill=0.0,
                base=63 - 2048 * m, channel_multiplier=64)
        nc.vector.memset(cf[0:2, 451:452], 1e-24)  # eps (sqrt bias)
        tmpb = cpool.tile([66, 1], DT, tag="tmpb", name="tmpb")
        nc.sync.dma_start(
            tmpb[:], w8[:, WROW:WROW + 66].rearrange("a b -> b a"))
        nc.vector.tensor_copy(cf[0:66, 450:451], tmpb[:])
        blkones_sb = cf[:, 0:2]
        sel26_sb = [cf[0:66, 2 + 128 * m:2 + 128 * (m + 1)] for m in range(MCH)]
        temp_sb = cf[0:66, 450:451]
        eps_sb = cf[0:2, 451:452]
        wot = cpool.tile([128, MCH * D], DT, tag="wot", name="wot")
        for s in range(FCH):
            nc.sync.dma_start(wot[:, s * DPC:(s + 1) * DPC],
                              wall[3 * D + s * 128:3 * D + (s + 1) * 128, :])
        woT_sb = [wot[:, bass.ts(i, D)] for i in range(MCH)]
        # temperature-scaled head-select matrices (for q's norm broadcast)
        sel26t_sb = []
        for m in range(MCH):
            t = cf[0:66, 452 + 128 * m:452 + 128 * (m + 1)]
            nc.vector.tensor_scalar_mul(t, sel26_sb[m], temp_sb)
            sel26t_sb.append(t)

        # persistent activations
        qT = [big.tile([128, S], DT, tag=f"qT{m}", name=f"qT{m}") for m in range(MCH)]
        kT = [big.tile([128, S], DT, tag=f"kT{m}", name=f"kT{m}") for m in range(MCH)]
        vaug2 = [big.tile([128, 8 * HPC * VW], DT, tag=f"vv{i}", name=f"vv{i}")
                 for i in range(2)]

        def vaug(t):
            return vaug2[t // 8][:, (t % 8) * HPC * VW:(t % 8 + 1) * HPC * VW]

        # ---------------- projections ----------------
        with tc.tile_pool(name="xin", bufs=1) as xin, \
             tc.tile_pool(name="win", bufs=1) as win, \
             tc.tile_pool(name="ptmp", bufs=1) as ptmp, \
             tc.tile_pool(name="pps", bufs=2, space="PSUM") as pps, \
             tc.tile_pool(name="pss", bufs=1, space="PSUM") as pss, \
             tc.tile_pool(name="pbv", bufs=2, space="PSUM") as pbv:

            xT_sb = []
            for i in range(FCH):
                t = xin.tile([128, S], DT, tag=f"x{i}", name=f"x{i}")
                for h in range(2):
                    nc.sync.dma_start(
                        t[:, h * SH:(h + 1) * SH],
                        xall[h * D + i * 128:h * D + (i + 1) * 128, :])
                xT_sb.append(t)

            def load_w(p):
                wfull = win.tile([128, FCH * DPC], DT, tag="wfull",
                                 name="wfull", bufs=1)
                base = {"q": 0, "k": D, "v": 2 * D}[p]
                for i in range(FCH):
                    nc.sync.dma_start(wfull[:, bass.ts(i, DPC)],
                                      wall[base + i * 128:base + (i + 1) * 128, :])
                return [wfull[:, bass.ts(i, DPC)] for i in range(FCH)]

            # ---- v: natural layout [s, d] with interleaved ones columns ----
            w_sb = load_w("v")
            nc.vector.memset(vaug2[0][:], 1.0)
            nc.vector.memset(vaug2[1][:], 1.0)
            for t in range(NB):
                ps = pps.tile([128, DPC], F32, tag="ps", name="ps")
                for kk in range(FCH):
                    nc.tensor.matmul(
                        ps[:], xT_sb[kk][:, t * 128:(t + 1) * 128], w_sb[kk],
                        start=(kk == 0), stop=(kk == FCH - 1))
                dst = vaug(t).rearrange("p (h e) -> p h e", e=VW)[:, :, 0:HD]
                src = ps[:].rearrange("p (h e) -> p h e", e=HD)
                nc.vector.tensor_copy(dst, src)

            # ---- q, k: transposed layout + cosine normalization ----
            for p, dst in (("q", qT), ("k", kT)):
                w_sb = load_w(p)
                # per-chunk norm rows live at partition 32*m (start-partition
                # must be a multiple of 32); filler rows are 1.0 so the
                # in-place reciprocal stays finite.
                norm = ptmp.tile([66, S], F32, tag="norm", name="norm")
                nc.vector.memset(norm[:], 1.0)
                raws = []
                for m in range(MCH):
                    raw = ptmp.tile([128, S], F32, tag=f"raw{m}", name=f"raw{m}")
                    raws.append(raw)
                    ss = pss.tile([2, S], F32, tag="ss", name="ss")
                    for g in range(NSEG):
                        sl = bass.ts(g, SEG)
                        ps = pps.tile([128, SEG], F32, tag="ps", name="ps")
                        for kk in range(FCH):
                            nc.tensor.matmul(
                                ps[:], w_sb[kk][:, m * 128:(m + 1) * 128],
                                xT_sb[kk][:, sl],
                                start=(kk == 0), stop=(kk == FCH - 1))
                        nc.vector.tensor_copy(raw[:, sl], ps[:])
                        sq = ptmp.tile([128, SEG], F32, tag="sq", name="sq")
                        nc.scalar.activation(sq[:], ps[:], AF.Square)
                        nc.tensor.matmul(ss[:, sl], blkones_sb, sq[:])
                    # ||row|| with eps clamp folded into sqrt bias
                    nc.scalar.activation(
                        norm[32 * m:32 * m + 2, :], ss[:], AF.Sqrt,
                        bias=eps_sb)
                nc.vector.reciprocal(norm[:], norm[:])
                inv = norm
                sel = sel26t_sb if p == "q" else sel26_sb
                for m in range(MCH):
                    for g in range(NSEG):
                        sl = bass.ts(g, SEG)
                        bc = pbv.tile([128, SEG], F32, tag="bc", name="bc")
                        nc.tensor.matmul(bc[:], sel[m], inv[:, sl])
                        nc.vector.tensor_tensor(
                            dst[m][:, sl], raws[m][:, sl], bc[:], MUL)

        # ---------------- attention ----------------
        aT = [big.tile([128, S], DT, tag=f"aT{m}", name=f"aT{m}") for m in range(MCH)]
        # causal masks generated on device: mask_d[p, f] = (p + 128d <= f)
        maskt = big.tile([128, NSEG * SEG], DT, tag="maskt", name="maskt")
        nc.vector.memset(maskt[:], 1.0)
        for d in range(NSEG):
            msl = maskt[:, d * SEG:(d + 1) * SEG]
            nc.gpsimd.affine_select(
                msl, msl, pattern=[[1, SEG]],
                compare_op=mybir.AluOpType.is_ge, fill=0.0,
                base=-128 * d, channel_multiplier=-1)
        masks_sb = [maskt[:, bass.ts(d, SEG)] for d in range(NSEG)]
        # Two heads per chunk emitted adjacently: their K=64 score matmuls
        # target disjoint row halves of the PE array (tile_position derives
        # from lhsT base partition) and run concurrently.
        with tc.tile_pool(name="attn", bufs=6) as apool, \
             tc.tile_pool(name="rpool", bufs=4) as rpool, \
             tc.tile_pool(name="psc", bufs=4, space="PSUM") as psc, \
             tc.tile_pool(name="pout", bufs=3, space="PSUM") as pout:
            for m in range(MCH):
                for c in range(NSEG):
                    csl = bass.ts(c, SEG)
                    nj = 4 * c + 4
                    ops = [pout.tile([VW, SEG], F32, tag="ops",
                                     name=f"ops{hh}") for hh in range(2)]
                    for j in range(nj):
                        ats = []
                        for hh in range(2):
                            hsl = slice(hh * 64, hh * 64 + 64)
                            sc = psc.tile([128, SEG], F32, tag="sc",
                                          name=f"sc{hh}")
                            nc.tensor.matmul(
                                sc[:], kT[m][hsl, j * 128:(j + 1) * 128],
                                qT[m][hsl, csl])
                            at = apool.tile([128, SEG], DT, tag="at",
                                            name=f"at{hh}")
                            nc.scalar.activation(at[:], sc[:], AF.Exp)
                            if j >= 4 * c:
                                nc.vector.tensor_tensor(
                                    at[:], at[:], masks_sb[j - 4 * c], MUL)
                            ats.append(at)
                        for hh in range(2):
                            h = 2 * m + hh
                            nc.tensor.matmul(
                                ops[hh][:], vaug(j)[:, h * VW:(h + 1) * VW],
                                ats[hh][:],
                                start=(j == 0), stop=(j == nj - 1))
                    for hh in range(2):
                        hsl = slice(hh * 64, hh * 64 + 64)
                        rec = rpool.tile([1, SEG], F32, tag="rec",
                                         name=f"rec{hh}")
                        nc.vector.reciprocal(rec[:], ops[hh][HD:HD + 1, :])
                        bcs = rpool.tile([HD, SEG], F32, tag="bcs",
                                         name=f"bcs{hh}")
                        nc.gpsimd.partition_broadcast(bcs[:], rec[:])
                        nc.vector.tensor_tensor(
                            aT[m][hsl, csl], ops[hh][0:HD, :], bcs[:], MUL)

        # ---------------- output projection (partial y to DRAM) ----------------
        ypart = dpool.tile([S, D], F32, tag="ypart", name="ypart")
        with tc.tile_pool(name="py", bufs=3, space="PSUM") as py, \
             tc.tile_pool(name="yout", bufs=3) as yout:
            for t in range(NB):
                yps = py.tile([128, D], F32, tag="y", name="y")
                for i in range(MCH):
                    for off, w in ((0, 512), (512, 256)):
                        nc.tensor.matmul(
                            yps[:, off:off + w],
                            aT[i][:, t * 128:(t + 1) * 128],
                            woT_sb[i][:, off:off + w],
                            start=(i == 0), stop=(i == MCH - 1))
                ysb = yout.tile([128, D], F32, tag="ysb", name="ysb")
                nc.vector.tensor_copy(ysb[:], yps[:])
                nc.sync.dma_start(ypart[t * 128:(t + 1) * 128, :], ysb[:])

        # ---------------- pair-sum + quantized output ----------------
        yhalf = dpool.tile([SH, D], F32, tag="yhalf", name="yhalf")
        nc.gpsimd.collective_compute(
            "ReduceScatter", mybir.AluOpType.add, replica_groups=P_GROUPS,
            ins=[ypart.opt()], outs=[yhalf.opt()])
        with tc.tile_pool(name="cva", bufs=3) as cva, \
             tc.tile_pool(name="cvo", bufs=3) as cvo, \
             tc.tile_pool(name="cvs", bufs=3) as cvs:
            for t in range(SH // 128):
                rs = slice(t * 128, (t + 1) * 128)
                a = cva.tile([128, D], F32, tag="a", name="a")
                nc.sync.dma_start(a[:], yhalf[rs, :])
                mx = cvs.tile([128, 4], F32, tag="mx", name="mx")
                nc.vector.reduce_max(mx[:, 0:1], a[:],
                                     axis=mybir.AxisListType.XYZW,
                                     apply_absolute_value=True)
                nc.vector.tensor_scalar_max(mx[:, 0:1], mx[:, 0:1], 1e-30)
                nc.vector.reciprocal(mx[:, 1:2], mx[:, 0:1])
                nc.vector.tensor_scalar_mul(mx[:, 2:3], mx[:, 1:2], 126.5)
                nc.vector.tensor_scalar_mul(mx[:, 3:4], mx[:, 0:1], 1.0 / 126.5)
                v = cvo.tile([128, D], F32, tag="v", name="v")
                nc.vector.tensor_scalar(v[:], a[:], mx[:, 2:3], 128.0,
                                        MUL, mybir.AluOpType.add)
                o = cvo.tile([128, D], mybir.dt.uint8, tag="o", name="o")
                nc.vector.tensor_copy(o[:], v[:])
                nc.sync.dma_start(y[rs, 0:D], o[:])
                nc.sync.dma_start(y_f32[rs, D // 4:D // 4 + 1], mx[:, 3:4])

    nc.compile()
    return nc


def _get_nc():
    if "nc" not in _CACHE:
        _CACHE["nc"] = _build_nc()
    return _CACHE["nc"]


def make_in_maps(x, Wq, Wk, Wv, Wo, temperature):
    x = np.asarray(x, np.float32)
    tv = np.asarray(temperature, np.float32).reshape(H)
    quarters, tmps = {}, {}
    for g in range(NGRP):
        hs = slice(g * DPC, (g + 1) * DPC)
        wq = np.asarray(Wq)[hs, :].T.astype(NPDT)            # [768, 384]
        wk = np.asarray(Wk)[hs, :].T.astype(NPDT)
        wv = np.asarray(Wv)[hs, :].T.astype(NPDT)
        woT = np.concatenate(
            np.asarray(Wo)[:, hs].T.astype(NPDT).reshape(MCH, 128, D),
            axis=1)                                          # [128, 2304]
        # pre-tile wo into 6 [128, 384] blocks so the device sees plain
        # 2D slices of the gathered [3072, 384] bundle
        wo_t = woT.reshape(128, FCH, DPC).transpose(1, 0, 2)
        quarters[g] = np.concatenate(
            [wq.reshape(-1), wk.reshape(-1), wv.reshape(-1),
             wo_t.reshape(-1)]).reshape(4, WROW)
        tp = np.ones(66, np.float32)
        for m in range(MCH):                # temp at rows 32m+k, matching sel26
            tp[32 * m] = tv[g * HPC + 2 * m]
            tp[32 * m + 1] = tv[g * HPC + 2 * m + 1]
        tmps[g] = tp.astype(NPDT)
    in_maps = []
    for c in range(NCORES):
        b, g = c // NGRP, c % NGRP
        xh = np.ascontiguousarray(x[b].T[:, g * SH:(g + 1) * SH].astype(NPDT))
        xch = D // XSPL
        m = {f"xh{i}": xh[i * xch:(i + 1) * xch, :] for i in range(XSPL)}
        m["w8"] = np.concatenate(
            [quarters[g][b], tmps[g]]).reshape(1, WROW + 66)
        in_maps.append(m)
    return in_maps


def kernel(x, Wq, Wk, Wv, Wo, temperature):
    nc = _get_nc()
    in_maps = make_in_maps(x, Wq, Wk, Wv, Wo, temperature)
    res = bass_utils.run_bass_kernel_spmd(
        nc, in_maps, core_ids=list(range(NCORES)))

    def dq(yq):
        sc = np.ascontiguousarray(yq[:, D:D + 4]).view(np.float32)
        return (yq[:, 0:D].astype(np.float32) - 128.0) * sc

    out = np.empty((B, S, D), np.float32)
    for b in range(B):
        out[b, 0:SH] = dq(res.results[NGRP * b]["y"])
        out[b, SH:S] = dq(res.results[NGRP * b + 1]["y"])
    return out
